# revision 38
# baseline (speedup 1.0000x reference)
"""Low-rank attention Trainium2 kernel, 8-way sharded (batch x head-group).

Reference computation (per full input):
  q = (hs @ Wq + bq).reshape(B,S,16,32); k likewise; v = (hs @ Wv + bv) -> (B,S,16,64)
  scores = einsum('bihr,bjhr->bijh', q, k) / sqrt(32); attn = softmax_j
  out = einsum('bijh,bjhd->bihd', attn, v).reshape(B,S,1024) @ Wo + bo

Sharding: core c = 4*b + g handles batch b and heads 4g..4g+3. Each core
computes its partial out-projection (256 rows of Wo); host sums the 4
partials per batch and adds bo.

On-device layout (per core): everything "feature-on-partition":
  XT   [1024->8x128 e, 2048]  bf16 (via cast-DMA + xbar transpose-DMA)
  QT/KT [128 (4h x 32r), 2048 i] bf16
  V_aug [128 j, 16 jt, 4 h, 65] bf16 (65th col = ones -> softmax denominator)
  scores^T [128 j, 1024 i-half] f32 PSUM -> exp (ACT, scale=1/sqrt(32)) -> bf16
  O^T accum [65, 512] PSUM (row 64 = denominator); normalize via reciprocal +
  gpsimd partition_broadcast + DVE multiply; out-proj from O^T chunks.
"""

import numpy as np

S = 2048
E = 1024
HL = 4          # local heads per core
R = 32
D = 64
DA = D + 1      # V augmented with ones column
NCORES = 8
SCALE = float(1.0 / np.sqrt(np.float32(R)))

_CACHE = {}


def _build_nc():
    from contextlib import ExitStack

    import concourse.bass as bass  # noqa: F401
    import concourse.tile as tile
    from concourse import bacc, mybir

    f32 = mybir.dt.float32
    bf16 = mybir.dt.bfloat16
    EXP = mybir.ActivationFunctionType.Exp

    nc = bacc.Bacc("TRN2", target_bir_lowering=False, debug=False,
                   num_devices=NCORES)

    x = nc.dram_tensor("x", [S, E], bf16, kind="ExternalInput").ap()
    wq = nc.dram_tensor("wq", [E, HL * R], bf16, kind="ExternalInput").ap()
    wk = nc.dram_tensor("wk", [E, HL * R], bf16, kind="ExternalInput").ap()
    wv = nc.dram_tensor("wv", [E, HL * D], bf16, kind="ExternalInput").ap()
    wo = nc.dram_tensor("wo", [HL * D, E], bf16, kind="ExternalInput").ap()
    bq = nc.dram_tensor("bq", [HL * R, 1], f32, kind="ExternalInput").ap()
    bk = nc.dram_tensor("bk", [HL * R, 1], f32, kind="ExternalInput").ap()
    bv = nc.dram_tensor("bv", [1, HL * D], bf16, kind="ExternalInput").ap()
    y = nc.dram_tensor("y", [S, E], f32, kind="ExternalOutput").ap()

    with tile.TileContext(nc) as tc, ExitStack() as ctx:
        const = ctx.enter_context(tc.tile_pool(name="const", bufs=1))
        sbig = ctx.enter_context(tc.tile_pool(name="sbig", bufs=1))
        es_pool = ctx.enter_context(tc.tile_pool(name="es", bufs=36))
        norm = ctx.enter_context(tc.tile_pool(name="norm", bufs=4))
        ypool = ctx.enter_context(tc.tile_pool(name="ypool", bufs=3))
        ps_sc = ctx.enter_context(tc.tile_pool(name="ps_sc", bufs=2, space="PSUM"))
        ps_wk = ctx.enter_context(tc.tile_pool(name="ps_wk", bufs=4, space="PSUM"))

        # ---------------- constants / weights ----------------
        wq_sb = const.tile([128, 8, 128], bf16, tag="wq")
        wk_sb = const.tile([128, 8, 128], bf16, tag="wk")
        wv_sb = const.tile([128, 8, 256], bf16, tag="wv")
        wo_sb = const.tile([128, 2, 1024], bf16, tag="wo")
        nc.gpsimd.dma_start(out=wq_sb[:], in_=wq.rearrange("(c p) n -> p c n", p=128))
        nc.gpsimd.dma_start(out=wk_sb[:], in_=wk.rearrange("(c p) n -> p c n", p=128))
        nc.gpsimd.dma_start(out=wv_sb[:], in_=wv.rearrange("(c p) n -> p c n", p=128))
        nc.gpsimd.dma_start(out=wo_sb[:], in_=wo.rearrange("(c p) n -> p c n", p=128))
        bq_sb = const.tile([128, 1], f32, tag="bq")
        bk_sb = const.tile([128, 1], f32, tag="bk")
        bv_sb = const.tile([1, 256], bf16, tag="bv")
        nc.gpsimd.dma_start(out=bq_sb[:], in_=bq[:, :])
        nc.gpsimd.dma_start(out=bk_sb[:], in_=bk[:, :])
        nc.gpsimd.dma_start(out=bv_sb[:], in_=bv[:, :])
        ones_col = const.tile([1, 128], bf16, tag="ones")
        nc.vector.memset(ones_col[:], 1.0)

        # PE warm-up: dense dummy matmuls (no DMA deps) raise the HAM clock
        # from 1.2 to 2.4 GHz before the real work arrives (~3.4us window).
        warm = const.tile([128, 512], bf16, tag="warm")
        nc.vector.memset(warm[:], 0.0)
        wps = ps_wk.tile([128, 512], f32, tag="pw", name="warmps")
        for i in range(20):
            nc.tensor.matmul(wps[:], warm[:, 0:128], warm[:],
                             start=True, stop=True, skip_group_check=True)

        # ---------------- X -> XT (bf16, transposed) ----------------
        # Flat full-bandwidth f32->bf16 cast per sequence half, then xbar
        # transposes at half granularity (alternating the two HWDGE engines)
        # so QT/KT matmuls and the attention sweep start after half the data.
        xt = []
        for e in range(8):
            xt.append(sbig.tile([128, S], bf16, tag=f"xt{e}", name=f"xt{e}"))
        for sh in range(2):
            hrows = slice(1024 * sh, 1024 * sh + 1024)
            for e in range(8):
                eng = nc.sync if e % 2 == 0 else nc.scalar
                eng.dma_start(out=xt[e][:, 1024 * sh:1024 * sh + 1024],
                              in_=x[hrows, 128 * e:128 * e + 128],
                              transpose=True)

        # ---------------- QT / KT projections (half-outer, e-outer) ---------
        # K-padded KT (kt_pad): head h's 32 KT rows sit at partition rows 32h
        # (aligned with QT's row layout) in pair slot h%2; other rows stay
        # zero, so a K=64 contraction of array strip h//2 against the full QT
        # strip picks out exactly head h at full array activity.
        qt_sb = sbig.tile([128, S], bf16, tag="qt")
        kt_pad = sbig.tile([128, 2, S], bf16, tag="ktp")
        nc.vector.memset(kt_pad[:], 0.0)

        def make_qkt(sh, qkacc=None):
            if qkacc is None:
                qacc_t = ps_sc.tile([128, 1024], f32, tag="sc", name=f"qacc{sh}")
                qacc = [qacc_t[:, 0:512], qacc_t[:, 512:1024]]
                kacc = [ps_wk.tile([128, 512], f32, tag="pw", name=f"kacc{sh}_{i}")
                        for i in range(2)]
            else:
                qacc, kacc = qkacc
            for e in range(8):
                for cc in range(2):
                    c = 2 * sh + cc
                    sl = slice(512 * c, 512 * c + 512)
                    nc.tensor.matmul(qacc[cc][:], wq_sb[:, e, :], xt[e][:, sl],
                                     start=(e == 0), stop=(e == 7))
                    nc.tensor.matmul(kacc[cc][:], wk_sb[:, e, :], xt[e][:, sl],
                                     start=(e == 0), stop=(e == 7))
            for cc in range(2):
                c = 2 * sh + cc
                sl = slice(512 * c, 512 * c + 512)
                nc.vector.tensor_scalar_add(qt_sb[:, sl], qacc[cc][:], bq_sb[:])
                for h in range(HL):
                    nc.vector.tensor_scalar_add(
                        kt_pad[32 * h:32 * h + 32, h % 2, sl],
                        kacc[cc][32 * h:32 * h + 32, :],
                        bk_sb[32 * h:32 * h + 32, :])

        make_qkt(0)

        # ---------------- V (padded to 128 cols: [V_h | ones | zeros]) ------
        # Full-width stationary operand keeps the PE array fully active during
        # the attention loop (HAM stays at K=8/8); zero columns are free.
        # V j-tiles are computed inside the first attention sweep (just ahead
        # of their first AV use) so their full-K matmuls overlap the exp flow.
        v_sb = sbig.tile([128, 16, HL, 128], bf16, tag="v")
        nc.vector.memset(v_sb[:], 0.0)
        nc.vector.memset(v_sb[:, :, :, D:DA], 1.0)

        def make_v(t, pv=None):
            if pv is None:
                pv = ps_wk.tile([128, 256], f32, tag="pw", name=f"pv{t}")[:]
            for e in range(8):
                nc.tensor.matmul(pv, xt[e][:, 128 * t:128 * t + 128],
                                 wv_sb[:, e, :], start=(e == 0), stop=False)
            nc.tensor.matmul(pv, ones_col[:], bv_sb[:], start=False, stop=True)
            nc.vector.tensor_copy(v_sb[:, t, :, 0:D], pv)

        # ---------------- attention ----------------
        # Pairs are (0,2) / (1,3): scores for both heads of a pair run as two
        # concurrent K=64 zero-padded matmuls at array row strips 0/64.
        o_sb = sbig.tile([128, 2, S], bf16, tag="o")  # [hd, wo-chunk, i]

        def scores_exp(ih, p, t):
            es_t = []
            for idx in range(2):
                st = ps_sc.tile([128, 1024], f32, tag="sc", name=f"st{ih}{p}{t}{idx}")
                for c in range(2):
                    isl = slice(1024 * ih + 512 * c, 1024 * ih + 512 * c + 512)
                    nc.tensor.matmul(
                        st[:, 512 * c:512 * c + 512],
                        kt_pad[64 * idx:64 * idx + 64, p, 128 * t:128 * t + 128],
                        qt_sb[64 * idx:64 * idx + 64, isl],
                        start=True, stop=True,
                        tile_position=(64 * idx, 0))
                ex = es_pool.tile([128, 1024], bf16, tag="es", name=f"es{ih}{p}{t}{idx}")
                nc.scalar.activation(ex[:], st[:], EXP, scale=SCALE)
                es_t.append(ex)
            return es_t

        def av_unit(p, t, es_t, o_acc):
            heads = (p, p + 2)
            for idx, h in enumerate(heads):
                for c in range(2):
                    nc.tensor.matmul(
                        o_acc[idx][c][:], v_sb[:, t, h, :],
                        es_t[idx][:, 512 * c:512 * c + 512],
                        start=(t == 0), stop=(t == 15))

        def outproj_isub(isub, tail=False):
            """Out-projection for i-block isub (needs o_sb of ih=isub//8
            complete). PSUM comes from the scores pool, which is idle when
            these run (during the last AV sweep / the tail)."""
            y_sb = ypool.tile([128, E], f32, tag="y", name=f"ysb{isub}")
            yp = ps_sc.tile([128, 1024], f32, tag="sc", name=f"yps{isub}")
            for eh in range(2):
                for pr in range(2):
                    nc.tensor.matmul(yp[:, 512 * eh:512 * eh + 512],
                                     o_sb[:, pr, 128 * isub:128 * isub + 128],
                                     wo_sb[:, pr, 512 * eh:512 * eh + 512],
                                     start=(pr == 0), stop=(pr == 1))
            cp = nc.any.tensor_copy if tail else nc.vector.tensor_copy
            cp(y_sb[:], yp[:])
            eng = nc.scalar if (tail and isub % 2) else nc.sync
            eng.dma_start(out=y[128 * isub:128 * isub + 128, :], in_=y_sb[:])

        # Software-pipelined emission: the AV sweep of pair k is interleaved
        # 1:1 with scores/exp of pair k+1, so ACT (the bottleneck) never waits
        # on a phase boundary. V and QKT(sh1) weave into the first sweep's
        # scores-only prologue (the pw pool is free until o_acc is allocated).
        seq = [(0, 0), (0, 1), (1, 0), (1, 1)]
        es_cur = []
        for t in range(8):
            es_cur.append(scores_exp(0, 0, t))
            make_v(t)
        qk1 = ([ps_wk.tile([128, 512], f32, tag="pw", name="q1a")[:],
                ps_wk.tile([128, 512], f32, tag="pw", name="q1b")[:]],
               [ps_wk.tile([128, 512], f32, tag="pw", name="k1a")[:],
                ps_wk.tile([128, 512], f32, tag="pw", name="k1b")[:]])
        make_qkt(1, qkacc=qk1)
        for t in range(8, 16):
            es_cur.append(scores_exp(0, 0, t))
            make_v(t)
        for k, (ih, p) in enumerate(seq):
            o_acc = [[ps_wk.tile([128, 512], f32, tag="pw",
                                 name=f"oacc{ih}_{p}_{i}_{c}")
                      for c in range(2)] for i in range(2)]
            es_next = []
            for t in range(16):
                av_unit(p, t, es_cur[t], o_acc)
                if k < 2:
                    es_next.append(scores_exp(*seq[k + 1], t))
                elif k == 2:
                    # half of (1,1)'s scores here, half just-in-time in sweep 3
                    if t % 2 == 0:
                        es_next.append(scores_exp(1, 1, t // 2))
                    elif t % 4 == 3:
                        outproj_isub(t // 4)        # ih0 isubs 0..3
                else:
                    if t < 8:
                        es_cur.append(scores_exp(1, 1, t + 8))
                    elif t % 2 == 1:
                        outproj_isub(4 + (t - 8) // 2)  # ih0 isubs 4..7
            es_cur = es_next
            # normalize: rows 0..63 = O^T unnorm, row 64 = denominator
            for idx, h in enumerate((p, p + 2)):
                for c in range(2):
                    acc = o_acc[idx][c]
                    rec = norm.tile([1, 512], f32, tag="rec")
                    nc.vector.reciprocal_approx_fast(rec[:], acc[D:DA, :])
                    rb = norm.tile([64, 512], f32, tag="rb")
                    nc.gpsimd.partition_broadcast(rb[:], rec[:])
                    isl = slice(1024 * ih + 512 * c, 1024 * ih + 512 * c + 512)
                    nc.vector.tensor_mul(
                        o_sb[64 * (h % 2):64 * (h % 2) + 64, h // 2, isl],
                        acc[0:D, :], rb[:])
        for isub in range(8, 16):
            outproj_isub(isub, tail=True)

    nc.compile()
    return nc


def _shard_inputs(inputs):
    """Build the 8 per-core input maps from full inputs (bf16 host-cast)."""
    import ml_dtypes

    bf = ml_dtypes.bfloat16
    hs = np.asarray(inputs["hidden_states"], dtype=np.float32).astype(bf)
    Wq = np.asarray(inputs["Wq"], dtype=np.float32).astype(bf)
    Wk = np.asarray(inputs["Wk"], dtype=np.float32).astype(bf)
    Wv = np.asarray(inputs["Wv"], dtype=np.float32).astype(bf)
    Wo = np.asarray(inputs["Wo"], dtype=np.float32).astype(bf)
    bq = np.asarray(inputs["bq"], dtype=np.float32)
    bk = np.asarray(inputs["bk"], dtype=np.float32)
    bv = np.asarray(inputs["bv"], dtype=np.float32).astype(bf)
    in_maps = []
    for core in range(NCORES):
        b, g = divmod(core, 4)
        qs = slice(128 * g, 128 * g + 128)
        vs = slice(256 * g, 256 * g + 256)
        in_maps.append({
            "x": np.ascontiguousarray(hs[b]),
            "wq": np.ascontiguousarray(Wq[:, qs]),
            "wk": np.ascontiguousarray(Wk[:, qs]),
            "wv": np.ascontiguousarray(Wv[:, vs]),
            "wo": np.ascontiguousarray(Wo[vs, :]),
            "bq": np.ascontiguousarray(bq[qs].reshape(128, 1)),
            "bk": np.ascontiguousarray(bk[qs].reshape(128, 1)),
            "bv": np.ascontiguousarray(bv[vs].reshape(1, 256)),
        })
    return in_maps


def kernel(**inputs) -> np.ndarray:
    from concourse.bass_utils import run_bass_kernel_spmd

    if "nc" not in _CACHE:
        _CACHE["nc"] = _build_nc()
    nc = _CACHE["nc"]

    in_maps = _shard_inputs(inputs)
    res = run_bass_kernel_spmd(nc, in_maps, core_ids=list(range(NCORES)))
    bo = np.asarray(inputs["bo"], dtype=np.float32)
    B = 2
    out = np.empty((B, S, E), dtype=np.float32)
    for b in range(B):
        acc = res.results[4 * b]["y"].astype(np.float32)
        for g in range(1, 4):
            acc = acc + res.results[4 * b + g]["y"]
        out[b] = acc + bo
    return out


# revision 39
# speedup vs baseline: 1.0724x; 1.0724x over previous
"""Low-rank attention Trainium2 kernel, 8-way sharded (batch x head-group).

Reference computation (per full input):
  q = (hs @ Wq + bq).reshape(B,S,16,32); k likewise; v = (hs @ Wv + bv) -> (B,S,16,64)
  scores = einsum('bihr,bjhr->bijh', q, k) / sqrt(32); attn = softmax_j
  out = einsum('bijh,bjhd->bihd', attn, v).reshape(B,S,1024) @ Wo + bo

Sharding: core c = 4*b + g handles batch b and heads 4g..4g+3. Each core
computes its partial out-projection (256 rows of Wo); host sums the 4
partials per batch and adds bo.

On-device layout (per core): everything "feature-on-partition":
  XT   [1024->8x128 e, 2048]  bf16 (via cast-DMA + xbar transpose-DMA)
  QT/KT [128 (4h x 32r), 2048 i] bf16
  V_aug [128 j, 16 jt, 4 h, 65] bf16 (65th col = ones -> softmax denominator)
  scores^T [128 j, 1024 i-half] f32 PSUM -> exp (ACT, scale=1/sqrt(32)) -> bf16
  O^T accum [65, 512] PSUM (row 64 = denominator); normalize via reciprocal +
  gpsimd partition_broadcast + DVE multiply; out-proj from O^T chunks.
"""

import numpy as np

S = 2048
E = 1024
HL = 4          # local heads per core
R = 32
D = 64
DA = D + 1      # V augmented with ones column
NCORES = 8
SCALE = float(1.0 / np.sqrt(np.float32(R)))

_CACHE = {}


def _build_nc():
    from contextlib import ExitStack

    import concourse.bass as bass  # noqa: F401
    import concourse.tile as tile
    from concourse import bacc, mybir

    f32 = mybir.dt.float32
    bf16 = mybir.dt.bfloat16
    EXP = mybir.ActivationFunctionType.Exp

    nc = bacc.Bacc("TRN2", target_bir_lowering=False, debug=False,
                   num_devices=NCORES)

    x = nc.dram_tensor("x", [S, E], bf16, kind="ExternalInput").ap()
    wq = nc.dram_tensor("wq", [E, HL * R], bf16, kind="ExternalInput").ap()
    wk = nc.dram_tensor("wk", [E, HL * R], bf16, kind="ExternalInput").ap()
    wv = nc.dram_tensor("wv", [E, HL * D], bf16, kind="ExternalInput").ap()
    wo = nc.dram_tensor("wo", [HL * D, E], bf16, kind="ExternalInput").ap()
    bq = nc.dram_tensor("bq", [HL * R, 1], f32, kind="ExternalInput").ap()
    bk = nc.dram_tensor("bk", [HL * R, 1], f32, kind="ExternalInput").ap()
    bv = nc.dram_tensor("bv", [1, HL * D], bf16, kind="ExternalInput").ap()
    y = nc.dram_tensor("y", [S, E], f32, kind="ExternalOutput").ap()

    with tile.TileContext(nc) as tc, ExitStack() as ctx:
        const = ctx.enter_context(tc.tile_pool(name="const", bufs=1))
        sbig = ctx.enter_context(tc.tile_pool(name="sbig", bufs=1))
        es_pool = ctx.enter_context(tc.tile_pool(name="es", bufs=36))
        norm = ctx.enter_context(tc.tile_pool(name="norm", bufs=4))
        ypool = ctx.enter_context(tc.tile_pool(name="ypool", bufs=3))
        ps_sc = ctx.enter_context(tc.tile_pool(name="ps_sc", bufs=2, space="PSUM"))
        ps_wk = ctx.enter_context(tc.tile_pool(name="ps_wk", bufs=4, space="PSUM"))

        # ---------------- constants / weights ----------------
        wq_sb = const.tile([128, 8, 128], bf16, tag="wq")
        wk_sb = const.tile([128, 8, 128], bf16, tag="wk")
        wv_sb = const.tile([128, 8, 256], bf16, tag="wv")
        wo_sb = const.tile([128, 2, 1024], bf16, tag="wo")
        nc.gpsimd.dma_start(out=wq_sb[:], in_=wq.rearrange("(c p) n -> p c n", p=128))
        nc.gpsimd.dma_start(out=wk_sb[:], in_=wk.rearrange("(c p) n -> p c n", p=128))
        nc.gpsimd.dma_start(out=wv_sb[:], in_=wv.rearrange("(c p) n -> p c n", p=128))
        nc.gpsimd.dma_start(out=wo_sb[:], in_=wo.rearrange("(c p) n -> p c n", p=128))
        bq_sb = const.tile([128, 1], f32, tag="bq")
        bk_sb = const.tile([128, 1], f32, tag="bk")
        bv_sb = const.tile([1, 256], bf16, tag="bv")
        nc.gpsimd.dma_start(out=bq_sb[:], in_=bq[:, :])
        nc.gpsimd.dma_start(out=bk_sb[:], in_=bk[:, :])
        nc.gpsimd.dma_start(out=bv_sb[:], in_=bv[:, :])
        ones_col = const.tile([1, 128], bf16, tag="ones")
        nc.vector.memset(ones_col[:], 1.0)


        # ---------------- X -> XT (bf16, transposed) ----------------
        # Flat full-bandwidth f32->bf16 cast per sequence half, then xbar
        # transposes at half granularity (alternating the two HWDGE engines)
        # so QT/KT matmuls and the attention sweep start after half the data.
        xt = []
        for e in range(8):
            xt.append(sbig.tile([128, S], bf16, tag=f"xt{e}", name=f"xt{e}"))
        for sh in range(2):
            hrows = slice(1024 * sh, 1024 * sh + 1024)
            for e in range(8):
                eng = nc.sync if e % 2 == 0 else nc.scalar
                eng.dma_start(out=xt[e][:, 1024 * sh:1024 * sh + 1024],
                              in_=x[hrows, 128 * e:128 * e + 128],
                              transpose=True)

        # ---------------- QT / KT projections (half-outer, e-outer) ---------
        # K-padded KT (kt_pad): head h's 32 KT rows sit at partition rows 32h
        # (aligned with QT's row layout) in pair slot h%2; other rows stay
        # zero, so a K=64 contraction of array strip h//2 against the full QT
        # strip picks out exactly head h at full array activity.
        qt_sb = sbig.tile([128, S], bf16, tag="qt")
        kt_pad = sbig.tile([128, 2, S], bf16, tag="ktp")
        nc.vector.memset(kt_pad[:], 0.0)

        def make_qkt(sh, qkacc=None):
            if qkacc is None:
                qacc_t = ps_sc.tile([128, 1024], f32, tag="sc", name=f"qacc{sh}")
                qacc = [qacc_t[:, 0:512], qacc_t[:, 512:1024]]
                kacc = [ps_wk.tile([128, 512], f32, tag="pw", name=f"kacc{sh}_{i}")
                        for i in range(2)]
            else:
                qacc, kacc = qkacc
            for e in range(8):
                for cc in range(2):
                    c = 2 * sh + cc
                    sl = slice(512 * c, 512 * c + 512)
                    nc.tensor.matmul(qacc[cc][:], wq_sb[:, e, :], xt[e][:, sl],
                                     start=(e == 0), stop=(e == 7))
                    nc.tensor.matmul(kacc[cc][:], wk_sb[:, e, :], xt[e][:, sl],
                                     start=(e == 0), stop=(e == 7))
            for cc in range(2):
                c = 2 * sh + cc
                sl = slice(512 * c, 512 * c + 512)
                nc.vector.tensor_scalar_add(qt_sb[:, sl], qacc[cc][:], bq_sb[:])
                for h in range(HL):
                    nc.vector.tensor_scalar_add(
                        kt_pad[32 * h:32 * h + 32, h % 2, sl],
                        kacc[cc][32 * h:32 * h + 32, :],
                        bk_sb[32 * h:32 * h + 32, :])

        make_qkt(0)

        # ---------------- V (padded to 128 cols: [V_h | ones | zeros]) ------
        # Full-width stationary operand keeps the PE array fully active during
        # the attention loop (HAM stays at K=8/8); zero columns are free.
        # V j-tiles are computed inside the first attention sweep (just ahead
        # of their first AV use) so their full-K matmuls overlap the exp flow.
        v_sb = sbig.tile([128, 16, HL, 128], bf16, tag="v")
        nc.vector.memset(v_sb[:], 0.0)
        nc.vector.memset(v_sb[:, :, :, D:DA], 1.0)

        def make_v(t, pv=None):
            if pv is None:
                pv = ps_wk.tile([128, 256], f32, tag="pw", name=f"pv{t}")[:]
            for e in range(8):
                nc.tensor.matmul(pv, xt[e][:, 128 * t:128 * t + 128],
                                 wv_sb[:, e, :], start=(e == 0), stop=False)
            nc.tensor.matmul(pv, ones_col[:], bv_sb[:], start=False, stop=True)
            nc.vector.tensor_copy(v_sb[:, t, :, 0:D], pv)

        # ---------------- attention ----------------
        # Pairs are (0,2) / (1,3): scores for both heads of a pair run as two
        # concurrent K=64 zero-padded matmuls at array row strips 0/64.
        o_sb = sbig.tile([128, 2, S], bf16, tag="o")  # [hd, wo-chunk, i]

        def scores_exp(ih, p, t):
            es_t = []
            for idx in range(2):
                st = ps_sc.tile([128, 1024], f32, tag="sc", name=f"st{ih}{p}{t}{idx}")
                for c in range(2):
                    isl = slice(1024 * ih + 512 * c, 1024 * ih + 512 * c + 512)
                    nc.tensor.matmul(
                        st[:, 512 * c:512 * c + 512],
                        kt_pad[64 * idx:64 * idx + 64, p, 128 * t:128 * t + 128],
                        qt_sb[64 * idx:64 * idx + 64, isl],
                        start=True, stop=True,
                        tile_position=(64 * idx, 0))
                ex = es_pool.tile([128, 1024], bf16, tag="es", name=f"es{ih}{p}{t}{idx}")
                nc.scalar.activation(ex[:], st[:], EXP, scale=SCALE)
                es_t.append(ex)
            return es_t

        def av_unit(p, t, es_t, o_acc):
            heads = (p, p + 2)
            for idx, h in enumerate(heads):
                for c in range(2):
                    nc.tensor.matmul(
                        o_acc[idx][c][:], v_sb[:, t, h, :],
                        es_t[idx][:, 512 * c:512 * c + 512],
                        start=(t == 0), stop=(t == 15))

        def outproj_isub(isub, tail=False):
            """Out-projection for i-block isub (needs o_sb of ih=isub//8
            complete). PSUM comes from the scores pool, which is idle when
            these run (during the last AV sweep / the tail)."""
            y_sb = ypool.tile([128, E], f32, tag="y", name=f"ysb{isub}")
            yp = ps_sc.tile([128, 1024], f32, tag="sc", name=f"yps{isub}")
            for eh in range(2):
                for pr in range(2):
                    nc.tensor.matmul(yp[:, 512 * eh:512 * eh + 512],
                                     o_sb[:, pr, 128 * isub:128 * isub + 128],
                                     wo_sb[:, pr, 512 * eh:512 * eh + 512],
                                     start=(pr == 0), stop=(pr == 1))
            cp = nc.any.tensor_copy if tail else nc.vector.tensor_copy
            cp(y_sb[:], yp[:])
            eng = nc.scalar if (tail and isub % 2) else nc.sync
            eng.dma_start(out=y[128 * isub:128 * isub + 128, :], in_=y_sb[:])

        # Software-pipelined emission: the AV sweep of pair k is interleaved
        # 1:1 with scores/exp of pair k+1, so ACT (the bottleneck) never waits
        # on a phase boundary. V and QKT(sh1) weave into the first sweep's
        # scores-only prologue (the pw pool is free until o_acc is allocated).
        seq = [(0, 0), (0, 1), (1, 0), (1, 1)]
        es_cur = []
        for t in range(8):
            es_cur.append(scores_exp(0, 0, t))
            make_v(t)
        qk1 = ([ps_wk.tile([128, 512], f32, tag="pw", name="q1a")[:],
                ps_wk.tile([128, 512], f32, tag="pw", name="q1b")[:]],
               [ps_wk.tile([128, 512], f32, tag="pw", name="k1a")[:],
                ps_wk.tile([128, 512], f32, tag="pw", name="k1b")[:]])
        make_qkt(1, qkacc=qk1)
        for t in range(8, 16):
            es_cur.append(scores_exp(0, 0, t))
            make_v(t)
        for k, (ih, p) in enumerate(seq):
            o_acc = [[ps_wk.tile([128, 512], f32, tag="pw",
                                 name=f"oacc{ih}_{p}_{i}_{c}")
                      for c in range(2)] for i in range(2)]
            es_next = []
            for t in range(16):
                av_unit(p, t, es_cur[t], o_acc)
                if k < 2:
                    es_next.append(scores_exp(*seq[k + 1], t))
                elif k == 2:
                    # half of (1,1)'s scores here, half just-in-time in sweep 3
                    if t % 2 == 0:
                        es_next.append(scores_exp(1, 1, t // 2))
                    elif t % 4 == 3:
                        outproj_isub(t // 4)        # ih0 isubs 0..3
                else:
                    if t < 8:
                        es_cur.append(scores_exp(1, 1, t + 8))
                    elif t % 2 == 1:
                        outproj_isub(4 + (t - 8) // 2)  # ih0 isubs 4..7
            es_cur = es_next
            # normalize: rows 0..63 = O^T unnorm, row 64 = denominator
            for idx, h in enumerate((p, p + 2)):
                for c in range(2):
                    acc = o_acc[idx][c]
                    rec = norm.tile([1, 512], f32, tag="rec")
                    nc.vector.reciprocal_approx_fast(rec[:], acc[D:DA, :])
                    rb = norm.tile([64, 512], f32, tag="rb")
                    nc.gpsimd.partition_broadcast(rb[:], rec[:])
                    isl = slice(1024 * ih + 512 * c, 1024 * ih + 512 * c + 512)
                    nc.vector.tensor_mul(
                        o_sb[64 * (h % 2):64 * (h % 2) + 64, h // 2, isl],
                        acc[0:D, :], rb[:])
        for isub in range(8, 16):
            outproj_isub(isub, tail=True)

    nc.compile()
    return nc


def _shard_inputs(inputs):
    """Build the 8 per-core input maps from full inputs (bf16 host-cast)."""
    import ml_dtypes

    bf = ml_dtypes.bfloat16
    hs = np.asarray(inputs["hidden_states"], dtype=np.float32).astype(bf)
    Wq = np.asarray(inputs["Wq"], dtype=np.float32).astype(bf)
    Wk = np.asarray(inputs["Wk"], dtype=np.float32).astype(bf)
    Wv = np.asarray(inputs["Wv"], dtype=np.float32).astype(bf)
    Wo = np.asarray(inputs["Wo"], dtype=np.float32).astype(bf)
    bq = np.asarray(inputs["bq"], dtype=np.float32)
    bk = np.asarray(inputs["bk"], dtype=np.float32)
    bv = np.asarray(inputs["bv"], dtype=np.float32).astype(bf)
    in_maps = []
    for core in range(NCORES):
        b, g = divmod(core, 4)
        qs = slice(128 * g, 128 * g + 128)
        vs = slice(256 * g, 256 * g + 256)
        in_maps.append({
            "x": np.ascontiguousarray(hs[b]),
            "wq": np.ascontiguousarray(Wq[:, qs]),
            "wk": np.ascontiguousarray(Wk[:, qs]),
            "wv": np.ascontiguousarray(Wv[:, vs]),
            "wo": np.ascontiguousarray(Wo[vs, :]),
            "bq": np.ascontiguousarray(bq[qs].reshape(128, 1)),
            "bk": np.ascontiguousarray(bk[qs].reshape(128, 1)),
            "bv": np.ascontiguousarray(bv[vs].reshape(1, 256)),
        })
    return in_maps


def kernel(**inputs) -> np.ndarray:
    from concourse.bass_utils import run_bass_kernel_spmd

    if "nc" not in _CACHE:
        _CACHE["nc"] = _build_nc()
    nc = _CACHE["nc"]

    in_maps = _shard_inputs(inputs)
    res = run_bass_kernel_spmd(nc, in_maps, core_ids=list(range(NCORES)))
    bo = np.asarray(inputs["bo"], dtype=np.float32)
    B = 2
    out = np.empty((B, S, E), dtype=np.float32)
    for b in range(B):
        acc = res.results[4 * b]["y"].astype(np.float32)
        for g in range(1, 4):
            acc = acc + res.results[4 * b + g]["y"]
        out[b] = acc + bo
    return out
